# revision 42
# baseline (speedup 1.0000x reference)
import sys
import numpy as np

for _p in ("/opt/trn_rl_repo", "/root/.axon_site/_ro/trn_rl_repo"):
    if _p not in sys.path:
        sys.path.append(_p)

import os
import ml_dtypes

BF = ml_dtypes.bfloat16

B, N, NODE, FE = 128, 100, 2, 128
NODE_SIZES = [2, 16, 32]
FN2_OUT = [14, 30, 1]
NCORES = 8
GPC = B // NCORES            # graphs per core = 16
COLS = GPC * N               # 1600
PAIRS = N * N                # 10000
UPG = 10                     # units (1000-col) per graph
UPB = GPC * UPG              # units per block = 160
CPB = UPB * 2                # 500-col chunks per block = 320

# Unit routing for the t1 leaky-relu. 'A' = A-decomposition route (fe1 is one
# matmul without the x1 part; DVE custom op adds the broadcast A term and
# applies the lrelu). 'E' = extra-matmul route (fe1 gets a second broadcast-AP
# matmul on PE; ACT applies the lrelu).
FRAC_E = float(os.environ.get("K_FRAC_E", 13 / 160))
FRAC_L1D = float(os.environ.get("K_FRAC_L1D", 0.0))   # l1 units on DVE (rest Pool)
T1_DELAY = int(os.environ.get("K_T1_DELAY", 2))      # chunks
FE2_DELAY = int(os.environ.get("K_FE2_DELAY", 2))    # units
TREE_DELAY = int(os.environ.get("K_TREE_DELAY", 4))  # units
FN_SPREAD = int(os.environ.get("K_FN_SPREAD", 6))    # fn steps per unit


def _frac_assign(frac, mark, other, n=UPB):
    out = []
    acc = 0.0
    for _ in range(n):
        acc += frac
        if acc >= 1.0:
            out.append(mark)
            acc -= 1.0
        else:
            out.append(other)
    return out


def round_fp32r(a):
    u = np.ascontiguousarray(np.asarray(a, np.float32)).view(np.uint32)
    low = u & np.uint32(0xFFF)
    base = u & np.uint32(0xFFFFF000)
    add = ((low > 0x800) | ((low == 0x800) & (((u >> 12) & 1) == 1))).astype(np.uint32) << 12
    return (base + add).view(np.float32)


_CACHE = {}


def _register_custom_ops():
    """Register the fused add-broadcast + leaky-relu custom DVE op. It reads
    PSUM once (in0), adds the SBUF broadcast term (in1) and applies
    max(q, 0.2q) internally -- legal where scalar_tensor_tensor (which would
    read PSUM twice) is not."""
    from concourse.dve_spec import Spec, Src0, Src1, C0, maxx, lower, _has_src1
    from concourse.dve_ops import (DveOp, OPS, CUSTOM_DVE_SPECS,
                                   _SUB_OPCODE_FOR_NAME, _CUSTOM_DVE_ROW_BASE)
    from concourse.dve_uop import DveOpSpec

    for op in OPS:
        if op.name == "T1ADDB_ANT":
            return op

    def _ref(in0, in1, c0, c1, c2):
        q = in0.astype(np.float32) + in1
        return np.maximum(q, q * c0)

    _q = Src0 + Src1
    spec = Spec(body=maxx(_q, _q * C0), reference=_ref)
    name = "T1ADDB_ANT"
    opcode = _CUSTOM_DVE_ROW_BASE + len(OPS)
    op = DveOp(name, spec, subdim=False, uops_sha={})
    _SUB_OPCODE_FOR_NAME[name] = opcode
    OPS.append(op)
    CUSTOM_DVE_SPECS[name] = spec
    for ver in ("v3", "v4"):
        dsx = DveOpSpec(name=name, opcode=opcode,
                        uops=lower(spec, ver=ver), rd1_en=_has_src1(spec))
        op.uops_sha[ver] = dsx.sha(ver)
    return op


# ---- packed constant layouts (shared between _build and _host_prep) ----
def _bf_pack_layout():
    cols = {}
    c = 0
    for i in range(3):
        d = NODE_SIZES[i]
        od = FN2_OUT[i]
        cols[f"s1b_{i}"] = (d + 2, c, 128); c += 128   # [W1b-perm; w1c; b1]
        cols[f"s1a_{i}"] = (d, c, 128); c += 128       # W1a-perm
        cols[f"fe2T{i}"] = (128, c, 128); c += 128
        cols[f"fn1xT{i}"] = (d, c, 128); c += 128
        cols[f"fn1avT{i}"] = (128, c, 128); c += 128
        cols[f"fn2T{i}"] = (128, c, od); c += od
    return cols, c


def _f32_pack_layout():
    cols = {}
    c = 0
    for i in range(3):
        od = FN2_OUT[i]
        cols[f"b2_{i}"] = (128, c, 1); c += 1
        cols[f"fb1_{i}"] = (128, c, 1); c += 1
        cols[f"fb2_{i}"] = (od, c, 1); c += 1
    return cols, c


def _build():
    import concourse.bacc as bacc
    import concourse.mybir as mybir
    import concourse.tile as tile

    F32 = mybir.dt.float32
    F32R = mybir.dt.float32r
    BF16 = mybir.dt.bfloat16
    AF = mybir.ActivationFunctionType
    ALU = mybir.AluOpType
    AX = mybir.AxisListType

    T1OP = _register_custom_ops()

    nc = bacc.Bacc("TRN2", target_bir_lowering=False, debug=False, num_devices=NCORES)

    bfl, nbf = _bf_pack_layout()
    f32l, nf32 = _f32_pack_layout()

    din = {}
    din["xt0"] = nc.dram_tensor("xt0", [3, COLS], BF16, kind="ExternalInput")
    din["nrmo"] = nc.dram_tensor("nrmo", [2 * GPC, PAIRS], BF16, kind="ExternalInput")
    din["bfpack"] = nc.dram_tensor("bfpack", [128, nbf], BF16, kind="ExternalInput")
    din["f32pack"] = nc.dram_tensor("f32pack", [128, nf32], F32, kind="ExternalInput")
    out_d = nc.dram_tensor("out", [1, GPC], F32, kind="ExternalOutput")

    T1R = _frac_assign(FRAC_E, "E", "A")
    L1A = _frac_assign(FRAC_L1D, "D", "P")

    with tile.TileContext(nc) as tc:
        with (
            tc.tile_pool(name="const", bufs=1) as cpool,
            tc.tile_pool(name="xp", bufs=1) as xpool,
            tc.tile_pool(name="m1", bufs=3) as mpool,
            tc.tile_pool(name="rep", bufs=3) as rpool,
            tc.tile_pool(name="asb", bufs=3) as apool_a,
            tc.tile_pool(name="t1p", bufs=8) as tpool,
            tc.tile_pool(name="avp", bufs=8) as apool,
            tc.tile_pool(name="q1p", bufs=3) as q1pool,
            tc.tile_pool(name="qb", bufs=3) as qbpool,
            tc.tile_pool(name="y1p", bufs=2) as y1pool,
            tc.tile_pool(name="ps1", bufs=3, space="PSUM") as ps1,
            tc.tile_pool(name="ps2", bufs=2, space="PSUM") as ps2,
            tc.tile_pool(name="pfp", bufs=1, space="PSUM") as pfp,
        ):
            XT0 = cpool.tile([3, COLS], BF16, tag="xt0", name="w_xt0")
            nc.sync.dma_start(XT0[:], din["xt0"].ap())
            BFP = cpool.tile([128, nbf], BF16, tag="bfp", name="w_bfp")
            nc.sync.dma_start(BFP[:], din["bfpack"].ap())
            # f32pack is only needed later; load it after the first graphs'
            # M1 DMAs are queued (SP queue is FIFO).
            F32P = cpool.tile([128, nf32], F32, tag="f32p", name="w_f32p")
            deferred_loads = [(F32P, din["f32pack"])]

            def Wv(name):
                if name in bfl:
                    r, c, w = bfl[name]
                    return BFP[0:r, c:c + w]
                r, c, w = f32l[name]
                return F32P[0:r, c:c + w]

            xt = XT0                        # [3, COLS] block-0 x (+ones row)
            ty = None
            for blk in range(3):
                d = NODE_SIZES[blk]
                od = FN2_OUT[blk]
                s1b = Wv(f"s1b_{blk}")
                s1a = Wv(f"s1a_{blk}")
                fe2T = Wv(f"fe2T{blk}")
                b2 = Wv(f"b2_{blk}")
                fn1avT, fn1xT = Wv(f"fn1avT{blk}"), Wv(f"fn1xT{blk}")
                fn2T = Wv(f"fn2T{blk}")
                fb1, fb2 = Wv(f"fb1_{blk}"), Wv(f"fb2_{blk}")
                # fe2/av2 run 2 units late, the j-sum tree later still, and fn
                # matmuls are drip-fed: every engine's in-order queue only sees
                # instructions whose inputs were produced long ago.
                pending_t1 = []
                pending_fe2 = []
                pending_trees = []
                pending_fn = []
                qbufs = {}

                if blk < 2:
                    nd = NODE_SIZES[blk + 1]
                    nod = FN2_OUT[blk]
                    xnext = xpool.tile([nd + 1, COLS], BF16, tag=f"x{blk + 1}")
                    # coords + ones rows below the y rows, ready immediately
                    nc.sync.dma_start(xnext[nod:nod + 3, :], XT0[:, :])
                else:
                    xnext = None
                    ty = xpool.tile([1, COLS], F32, tag="ty")

                def emit_t1(p1_, t1h_, asb_, kc_, T1OP=T1OP, d=d):
                    ku = kc_ // 2
                    i0 = (kc_ % 2) * 5
                    if T1R[ku % UPB] == "E":
                        nc.scalar.activation(t1h_, p1_[:, 0:500], AF.Prelu,
                                             scale=1.0, alpha=0.2)
                    else:
                        p1v = p1_[:, 0:500].rearrange("p (a b) -> p a b", b=N)
                        t1v = t1h_.rearrange("p (a b) -> p a b", b=N)
                        uu = (kc_ // 2) % UPG
                        ii = uu * UPG + i0
                        av = asb_[:, ii:ii + 5].unsqueeze(2).broadcast_to([128, 5, N])
                        nc.vector._custom_dve(T1OP, out=t1v, in0=p1v, in1=av,
                                              s0=0.2)

                def emit_fe2(t1_, g_, u_, fe2T=fe2T, b2=b2):
                    p2 = ps2.tile([128, 1024], F32, tag="p2")
                    for ci in range(2):
                        nc.tensor.matmul(p2[:, ci * 512: ci * 512 + 500],
                                         fe2T, t1_[:, ci * 500:(ci + 1) * 500],
                                         start=True, stop=True)
                    p2v = p2[:].rearrange("p (a b) -> p a b", b=512)[:, :, 0:500]
                    av2 = apool.tile([128, 1000], BF16, tag="av2")
                    av2v = av2[:].rearrange("p (a b) -> p a b", b=500)
                    nc.scalar.activation(av2v, p2v, AF.Prelu,
                                         bias=b2, scale=1.0, alpha=0.2)
                    pending_trees.append((av2, g_, u_))
                    if len(pending_trees) > TREE_DELAY:
                        emit_tree(*pending_trees.pop(0))

                def emit_tree(av2_, g_, u_):
                    # one bf16 half-add (SBUF-only, so Pool can own it); the
                    # remaining 50-way sum per node is folded into the fn1av
                    # matmul (see fn steps below).
                    ku = g_ * UPG + u_
                    a3 = av2_[:].rearrange("p (a b) -> p a b", b=N)
                    pr = g_ // 2
                    if pr not in qbufs:
                        qbufs[pr] = qbpool.tile([128, 10000], BF16, tag="qb",
                                                name=f"qb{pr % 3}")
                    base = (g_ % 2) * 5000 + u_ * 500
                    q1v = qbufs[pr][:, base:base + 500].rearrange(
                        "p (a b) -> p a b", b=50)
                    e1 = nc.vector if L1A[ku % UPB] == "D" else nc.gpsimd
                    e1.tensor_tensor(q1v, a3[:, :, 0:50], a3[:, :, 50:100],
                                     ALU.add)

                def fn_pair_steps(g0, Qbuf, xnext=None, blk=blk, d=d, od=od,
                                  fn1avT=fn1avT, fn1xT=fn1xT, fn2T=fn2T,
                                  fb1=fb1, fb2=fb2):
                    csl = slice(g0 * N, (g0 + 2) * N)
                    state = {}
                    steps = []

                    def mk_start():
                        state["pf"] = pfp.tile([128, 512], F32, tag="pf",
                                               name="pf")
                        state["Qv"] = Qbuf[:].rearrange("p (a b) -> p a b", b=50)
                        nc.tensor.matmul(state["pf"][:, 0:200], fn1avT,
                                         state["Qv"][:, :, 0:1],
                                         start=True, stop=False,
                                         skip_group_check=True)
                    steps.append(mk_start)
                    for c in range(1, 50):
                        def mk_mm(c=c):
                            nc.tensor.matmul(state["pf"][:, 0:200], fn1avT,
                                             state["Qv"][:, :, c:c + 1],
                                             start=False, stop=False,
                                             skip_group_check=True)
                        steps.append(mk_mm)

                    def mk_tail():
                        nc.tensor.matmul(state["pf"][:, 0:200], fn1xT,
                                         xt[0:d, csl],
                                         start=False, stop=True,
                                         skip_group_check=True)
                        y1 = y1pool.tile([128, 200], BF16, tag="y1")
                        nc.scalar.activation(y1[:], state["pf"][:, 0:200],
                                             AF.Tanh, bias=fb1)
                        pf2 = pfp.tile([od, 512], F32, tag="pf", name="pf2")
                        nc.tensor.matmul(pf2[:, 0:200], fn2T, y1[:],
                                         start=True, stop=True)
                        if blk < 2:
                            nc.scalar.activation(xnext[0:od, csl],
                                                 pf2[:, 0:200], AF.Tanh,
                                                 bias=fb2)
                        else:
                            nc.scalar.activation(ty[:, csl], pf2[:, 0:200],
                                                 AF.Tanh, bias=fb2)
                    steps.append(mk_tail)
                    return steps

                def build_m1(g_):
                    gsl_ = slice(g_ * N, (g_ + 1) * N)
                    # M1 = [x2-tiled; nrm; ones]  [d+2, PAIRS] bf16 -- all
                    # transfers have contiguous fastest-moving dims.
                    rep4 = rpool.tile([d, 400], BF16, tag="rep4")
                    nc.sync.dma_start(
                        rep4[:].rearrange("p (a b) -> p a b", a=4),
                        xt[0:d, gsl_].unsqueeze(1).broadcast_to([d, 4, N]),
                    )
                    M1 = mpool.tile([d + 2, PAIRS], BF16, tag="m1")
                    nc.sync.dma_start(
                        M1[0:d, :].rearrange("p (a b) -> p a b", a=25),
                        rep4[:].unsqueeze(1).broadcast_to([d, 25, 400]),
                    )
                    nc.sync.dma_start(
                        M1[d:d + 2, :],
                        din["nrmo"].ap()[2 * g_:2 * g_ + 2, :],
                    )
                    # A = W1a^T @ x  [128, 100] -- the x1 ("i") part of fe1,
                    # added per-node by the custom DVE op.
                    pa = ps2.tile([128, 512], F32, tag="p2", name="pa")
                    nc.tensor.matmul(pa[:, 0:100], s1a, xt[0:d, gsl_],
                                     start=True, stop=True)
                    asb = apool_a.tile([128, 100], F32, tag="asb")
                    nc.scalar.activation(asb[:], pa[:, 0:100], AF.Copy)
                    return M1, asb

                m1q = [build_m1(0), build_m1(1)]
                while deferred_loads:
                    tile_, dram_ = deferred_loads.pop(0)
                    nc.sync.dma_start(tile_[:], dram_.ap())

                for g in range(GPC):
                    M1, asb = m1q.pop(0)
                    if g + 2 < GPC:
                        m1q.append(build_m1(g + 2))

                    for u in range(UPG):
                        t1 = tpool.tile([128, 1000], BF16, tag="t1")
                        route_e = T1R[(g * UPG + u) % UPB] == "E"
                        for ci in range(2):
                            kc = (g * UPG + u) * 2 + ci
                            c0 = u * 1000 + ci * 500
                            i0 = u * UPG + ci * 5
                            p1 = ps1.tile([128, 512], F32, tag="p1")
                            nc.tensor.matmul(p1[:, 0:500], s1b,
                                             M1[:, c0:c0 + 500],
                                             start=True, stop=not route_e)
                            if route_e:
                                mv = xt[0:d, g * N + i0: g * N + i0 + 5] \
                                    .unsqueeze(2).broadcast_to([d, 5, N])
                                nc.tensor.matmul(p1[:, 0:500], s1a, mv,
                                                 start=False, stop=True)
                            pending_t1.append(
                                (p1, t1[:, ci * 500:(ci + 1) * 500], asb, kc))
                            if len(pending_t1) > T1_DELAY:
                                emit_t1(*pending_t1.pop(0))
                        pending_fe2.append((t1, g, u))
                        if len(pending_fe2) > FE2_DELAY:
                            emit_fe2(*pending_fe2.pop(0))
                        for _ in range(FN_SPREAD):
                            if pending_fn:
                                pending_fn.pop(0)()

                    if g % 2 == 1 and g >= 3:
                        pending_fn.extend(fn_pair_steps(
                            g - 3, qbufs.pop((g - 3) // 2), xnext=xnext))

                while pending_t1:
                    emit_t1(*pending_t1.pop(0))
                while pending_fe2:
                    emit_fe2(*pending_fe2.pop(0))
                while pending_trees:
                    emit_tree(*pending_trees.pop(0))
                while pending_fn:
                    pending_fn.pop(0)()
                for step in fn_pair_steps(GPC - 2, qbufs.pop((GPC - 2) // 2),
                                          xnext=xnext):
                    step()
                if blk < 2:
                    xt = xnext

            # ---------- final: sigmoid(mean over N) ----------
            red = xpool.tile([1, GPC], F32, tag="red")
            nc.vector.tensor_reduce(red[:], ty[:].rearrange("p (a b) -> p a b", a=GPC),
                                    axis=AX.X, op=ALU.add)
            osb = xpool.tile([1, GPC], F32, tag="osb")
            nc.scalar.activation(osb[:], red[:], AF.Sigmoid, scale=1.0 / N)
            nc.sync.dma_start(out_d.ap(), osb[:])

    nc.compile()
    return nc


def _host_prep(inputs):
    """Build per-core in_maps from full inputs."""
    x = np.asarray(inputs["x"], np.float32)          # [B, N, 2]

    bfl, nbf = _bf_pack_layout()
    f32l, nf32 = _f32_pack_layout()
    bfpack = np.zeros((128, nbf), np.float32)
    f32pack = np.zeros((128, nf32), np.float32)

    def put(name, arr):
        arr = np.asarray(arr, np.float32)
        for lay, pack in ((bfl, bfpack), (f32l, f32pack)):
            if name in lay:
                r, c, w = lay[name]
                assert arr.shape == (r, w), (name, arr.shape, (r, w))
                pack[0:r, c:c + w] = arr
                return
        raise KeyError(name)

    for i in range(3):
        d = NODE_SIZES[i]
        fe1w = np.asarray(inputs[f"fe1w{i}"], np.float32)   # [128, 2d+1]
        fe1b = np.asarray(inputs[f"fe1b{i}"], np.float32)
        fe2w = np.asarray(inputs[f"fe2w{i}"], np.float32)   # [128, 128]
        fe2b = np.asarray(inputs[f"fe2b{i}"], np.float32)
        fn1w = np.asarray(inputs[f"fn1w{i}"], np.float32)   # [128, 128+d]
        fn1b = np.asarray(inputs[f"fn1b{i}"], np.float32)
        fn2w = np.asarray(inputs[f"fn2w{i}"], np.float32)   # [od, 128]
        fn2b = np.asarray(inputs[f"fn2b{i}"], np.float32)
        if i == 0:
            perm = np.arange(d)
        else:
            # my x row order [y..., c0, c1] -> ref order [c0, c1, y...]
            perm = np.concatenate([np.arange(2, d), [0, 1]])
        W1a = fe1w[:, 0:d][:, perm].T                       # [d, 128]
        W1b = fe1w[:, d:2 * d][:, perm].T                   # [d, 128]
        s1b = np.concatenate(
            [W1b, fe1w[:, 2 * d].reshape(1, 128), fe1b.reshape(1, 128)], axis=0
        )
        put(f"s1b_{i}", s1b)
        put(f"s1a_{i}", W1a)
        put(f"fe2T{i}", fe2w.T)
        put(f"fn1xT{i}", fn1w[:, 128:][:, perm].T)
        put(f"fn1avT{i}", fn1w[:, :128].T)
        put(f"fn2T{i}", fn2w.T)
        put(f"b2_{i}", fe2b.reshape(128, 1))
        put(f"fb1_{i}", fn1b.reshape(128, 1))
        put(f"fb2_{i}", fn2b.reshape(FN2_OUT[i], 1))

    shared = {
        "bfpack": bfpack.astype(BF),
        "f32pack": f32pack,
    }

    in_maps = []
    for c in range(NCORES):
        xf = x[c * GPC:(c + 1) * GPC]                        # [16, 100, 2]
        xt0 = np.concatenate(
            [xf.transpose(2, 0, 1).reshape(2, COLS), np.ones((1, COLS), np.float32)],
            axis=0,
        )
        diff = xf[:, :, None, :] - xf[:, None, :, :]
        nrm = np.sqrt((diff * diff).sum(-1)).reshape(GPC, PAIRS)
        nrmo = np.empty((2 * GPC, PAIRS), np.float32)
        nrmo[0::2] = nrm
        nrmo[1::2] = 1.0
        m = dict(shared)
        m["xt0"] = xt0.astype(BF)
        m["nrmo"] = nrmo.astype(BF)
        in_maps.append(m)
    return in_maps


def kernel(**inputs):
    from concourse import bass_utils

    if "nc" not in _CACHE:
        _CACHE["nc"] = _build()
    nc = _CACHE["nc"]
    in_maps = _host_prep(inputs)
    res = bass_utils.run_bass_kernel_spmd(nc, in_maps, core_ids=list(range(NCORES)))
    out = np.concatenate(
        [np.asarray(res.results[c]["out"], np.float32).reshape(GPC, 1) for c in range(NCORES)],
        axis=0,
    )
    return out


# revision 43
# speedup vs baseline: 1.0127x; 1.0127x over previous
import sys
import numpy as np

for _p in ("/opt/trn_rl_repo", "/root/.axon_site/_ro/trn_rl_repo"):
    if _p not in sys.path:
        sys.path.append(_p)

import os
import ml_dtypes

BF = ml_dtypes.bfloat16

B, N, NODE, FE = 128, 100, 2, 128
NODE_SIZES = [2, 16, 32]
FN2_OUT = [14, 30, 1]
NCORES = 8
GPC = B // NCORES            # graphs per core = 16
COLS = GPC * N               # 1600
PAIRS = N * N                # 10000
UPG = 10                     # units (1000-col) per graph
UPB = GPC * UPG              # units per block = 160
CPB = UPB * 2                # 500-col chunks per block = 320

# Unit routing for the t1 leaky-relu. 'A' = A-decomposition route (fe1 is one
# matmul without the x1 part; DVE custom op adds the broadcast A term and
# applies the lrelu). 'E' = extra-matmul route (fe1 gets a second broadcast-AP
# matmul on PE; ACT applies the lrelu).
FRAC_E = float(os.environ.get("K_FRAC_E", 8 / 160))
FRAC_L1D = float(os.environ.get("K_FRAC_L1D", 20 / 160))  # l1 units on DVE (rest Pool)
T1_DELAY = int(os.environ.get("K_T1_DELAY", 2))      # chunks
FE2_DELAY = int(os.environ.get("K_FE2_DELAY", 2))    # units
TREE_DELAY = int(os.environ.get("K_TREE_DELAY", 4))  # units
FN_SPREAD = int(os.environ.get("K_FN_SPREAD", 6))    # fn steps per unit


def _frac_assign(frac, mark, other, n=UPB):
    out = []
    acc = 0.0
    for _ in range(n):
        acc += frac
        if acc >= 1.0:
            out.append(mark)
            acc -= 1.0
        else:
            out.append(other)
    return out


def round_fp32r(a):
    u = np.ascontiguousarray(np.asarray(a, np.float32)).view(np.uint32)
    low = u & np.uint32(0xFFF)
    base = u & np.uint32(0xFFFFF000)
    add = ((low > 0x800) | ((low == 0x800) & (((u >> 12) & 1) == 1))).astype(np.uint32) << 12
    return (base + add).view(np.float32)


_CACHE = {}


def _register_custom_ops():
    """Register the fused add-broadcast + leaky-relu custom DVE op. It reads
    PSUM once (in0), adds the SBUF broadcast term (in1) and applies
    max(q, 0.2q) internally -- legal where scalar_tensor_tensor (which would
    read PSUM twice) is not."""
    from concourse.dve_spec import Spec, Src0, Src1, C0, maxx, lower, _has_src1
    from concourse.dve_ops import (DveOp, OPS, CUSTOM_DVE_SPECS,
                                   _SUB_OPCODE_FOR_NAME, _CUSTOM_DVE_ROW_BASE)
    from concourse.dve_uop import DveOpSpec

    for op in OPS:
        if op.name == "T1ADDB_ANT":
            return op

    def _ref(in0, in1, c0, c1, c2):
        q = in0.astype(np.float32) + in1
        return np.maximum(q, q * c0)

    _q = Src0 + Src1
    spec = Spec(body=maxx(_q, _q * C0), reference=_ref)
    name = "T1ADDB_ANT"
    opcode = _CUSTOM_DVE_ROW_BASE + len(OPS)
    op = DveOp(name, spec, subdim=False, uops_sha={})
    _SUB_OPCODE_FOR_NAME[name] = opcode
    OPS.append(op)
    CUSTOM_DVE_SPECS[name] = spec
    for ver in ("v3", "v4"):
        dsx = DveOpSpec(name=name, opcode=opcode,
                        uops=lower(spec, ver=ver), rd1_en=_has_src1(spec))
        op.uops_sha[ver] = dsx.sha(ver)
    return op


# ---- packed constant layouts (shared between _build and _host_prep) ----
def _bf_pack_layout():
    cols = {}
    c = 0
    for i in range(3):
        d = NODE_SIZES[i]
        od = FN2_OUT[i]
        cols[f"s1b_{i}"] = (d + 2, c, 128); c += 128   # [W1b-perm; w1c; b1]
        cols[f"s1a_{i}"] = (d, c, 128); c += 128       # W1a-perm
        cols[f"fe2T{i}"] = (128, c, 128); c += 128
        cols[f"fn1xT{i}"] = (d, c, 128); c += 128
        cols[f"fn1avT{i}"] = (128, c, 128); c += 128
        cols[f"fn2T{i}"] = (128, c, od); c += od
    return cols, c


def _f32_pack_layout():
    cols = {}
    c = 0
    for i in range(3):
        od = FN2_OUT[i]
        cols[f"b2_{i}"] = (128, c, 1); c += 1
        cols[f"fb1_{i}"] = (128, c, 1); c += 1
        cols[f"fb2_{i}"] = (od, c, 1); c += 1
    return cols, c


def _build():
    import concourse.bacc as bacc
    import concourse.mybir as mybir
    import concourse.tile as tile

    F32 = mybir.dt.float32
    F32R = mybir.dt.float32r
    BF16 = mybir.dt.bfloat16
    AF = mybir.ActivationFunctionType
    ALU = mybir.AluOpType
    AX = mybir.AxisListType

    T1OP = _register_custom_ops()

    nc = bacc.Bacc("TRN2", target_bir_lowering=False, debug=False, num_devices=NCORES)

    bfl, nbf = _bf_pack_layout()
    f32l, nf32 = _f32_pack_layout()

    din = {}
    din["xt0"] = nc.dram_tensor("xt0", [3, COLS], BF16, kind="ExternalInput")
    din["nrmo"] = nc.dram_tensor("nrmo", [2 * GPC, PAIRS], BF16, kind="ExternalInput")
    din["bfpack"] = nc.dram_tensor("bfpack", [128, nbf], BF16, kind="ExternalInput")
    din["f32pack"] = nc.dram_tensor("f32pack", [128, nf32], F32, kind="ExternalInput")
    out_d = nc.dram_tensor("out", [1, GPC], F32, kind="ExternalOutput")

    T1R = _frac_assign(FRAC_E, "E", "A")
    L1A = _frac_assign(FRAC_L1D, "D", "P")

    with tile.TileContext(nc) as tc:
        with (
            tc.tile_pool(name="const", bufs=1) as cpool,
            tc.tile_pool(name="xp", bufs=1) as xpool,
            tc.tile_pool(name="m1", bufs=3) as mpool,
            tc.tile_pool(name="rep", bufs=3) as rpool,
            tc.tile_pool(name="asb", bufs=3) as apool_a,
            tc.tile_pool(name="t1p", bufs=8) as tpool,
            tc.tile_pool(name="avp", bufs=8) as apool,
            tc.tile_pool(name="q1p", bufs=3) as q1pool,
            tc.tile_pool(name="qb", bufs=3) as qbpool,
            tc.tile_pool(name="y1p", bufs=2) as y1pool,
            tc.tile_pool(name="ps1", bufs=3, space="PSUM") as ps1,
            tc.tile_pool(name="ps2", bufs=2, space="PSUM") as ps2,
            tc.tile_pool(name="pfp", bufs=1, space="PSUM") as pfp,
        ):
            XT0 = cpool.tile([3, COLS], BF16, tag="xt0", name="w_xt0")
            nc.sync.dma_start(XT0[:], din["xt0"].ap())
            BFP = cpool.tile([128, nbf], BF16, tag="bfp", name="w_bfp")
            nc.sync.dma_start(BFP[:], din["bfpack"].ap())
            # f32pack is only needed later; load it after the first graphs'
            # M1 DMAs are queued (SP queue is FIFO).
            F32P = cpool.tile([128, nf32], F32, tag="f32p", name="w_f32p")
            deferred_loads = [(F32P, din["f32pack"])]

            def Wv(name):
                if name in bfl:
                    r, c, w = bfl[name]
                    return BFP[0:r, c:c + w]
                r, c, w = f32l[name]
                return F32P[0:r, c:c + w]

            xt = XT0                        # [3, COLS] block-0 x (+ones row)
            ty = None
            for blk in range(3):
                d = NODE_SIZES[blk]
                od = FN2_OUT[blk]
                s1b = Wv(f"s1b_{blk}")
                s1a = Wv(f"s1a_{blk}")
                fe2T = Wv(f"fe2T{blk}")
                b2 = Wv(f"b2_{blk}")
                fn1avT, fn1xT = Wv(f"fn1avT{blk}"), Wv(f"fn1xT{blk}")
                fn2T = Wv(f"fn2T{blk}")
                fb1, fb2 = Wv(f"fb1_{blk}"), Wv(f"fb2_{blk}")
                # fe2/av2 run 2 units late, the j-sum tree later still, and fn
                # matmuls are drip-fed: every engine's in-order queue only sees
                # instructions whose inputs were produced long ago.
                pending_t1 = []
                pending_fe2 = []
                pending_trees = []
                pending_fn = []
                qbufs = {}

                if blk < 2:
                    nd = NODE_SIZES[blk + 1]
                    nod = FN2_OUT[blk]
                    xnext = xpool.tile([nd + 1, COLS], BF16, tag=f"x{blk + 1}")
                    # coords + ones rows below the y rows, ready immediately
                    nc.sync.dma_start(xnext[nod:nod + 3, :], XT0[:, :])
                else:
                    xnext = None
                    ty = xpool.tile([1, COLS], F32, tag="ty")

                def emit_t1(p1_, t1h_, asb_, kc_, T1OP=T1OP, d=d):
                    ku = kc_ // 2
                    i0 = (kc_ % 2) * 5
                    if T1R[ku % UPB] == "E":
                        nc.scalar.activation(t1h_, p1_[:, 0:500], AF.Prelu,
                                             scale=1.0, alpha=0.2)
                    else:
                        p1v = p1_[:, 0:500].rearrange("p (a b) -> p a b", b=N)
                        t1v = t1h_.rearrange("p (a b) -> p a b", b=N)
                        uu = (kc_ // 2) % UPG
                        ii = uu * UPG + i0
                        av = asb_[:, ii:ii + 5].unsqueeze(2).broadcast_to([128, 5, N])
                        nc.vector._custom_dve(T1OP, out=t1v, in0=p1v, in1=av,
                                              s0=0.2)

                def emit_fe2(t1_, g_, u_, fe2T=fe2T, b2=b2):
                    p2 = ps2.tile([128, 1024], F32, tag="p2")
                    for ci in range(2):
                        nc.tensor.matmul(p2[:, ci * 512: ci * 512 + 500],
                                         fe2T, t1_[:, ci * 500:(ci + 1) * 500],
                                         start=True, stop=True)
                    p2v = p2[:].rearrange("p (a b) -> p a b", b=512)[:, :, 0:500]
                    av2 = apool.tile([128, 1000], BF16, tag="av2")
                    av2v = av2[:].rearrange("p (a b) -> p a b", b=500)
                    nc.scalar.activation(av2v, p2v, AF.Prelu,
                                         bias=b2, scale=1.0, alpha=0.2)
                    pending_trees.append((av2, g_, u_))
                    if len(pending_trees) > TREE_DELAY:
                        emit_tree(*pending_trees.pop(0))

                def emit_tree(av2_, g_, u_):
                    # one bf16 half-add (SBUF-only, so Pool can own it); the
                    # remaining 50-way sum per node is folded into the fn1av
                    # matmul (see fn steps below).
                    ku = g_ * UPG + u_
                    a3 = av2_[:].rearrange("p (a b) -> p a b", b=N)
                    pr = g_ // 2
                    if pr not in qbufs:
                        qbufs[pr] = qbpool.tile([128, 10000], BF16, tag="qb",
                                                name=f"qb{pr % 3}")
                    base = (g_ % 2) * 5000 + u_ * 500
                    q1v = qbufs[pr][:, base:base + 500].rearrange(
                        "p (a b) -> p a b", b=50)
                    e1 = nc.vector if L1A[ku % UPB] == "D" else nc.gpsimd
                    e1.tensor_tensor(q1v, a3[:, :, 0:50], a3[:, :, 50:100],
                                     ALU.add)

                def fn_pair_steps(g0, Qbuf, xnext=None, blk=blk, d=d, od=od,
                                  fn1avT=fn1avT, fn1xT=fn1xT, fn2T=fn2T,
                                  fb1=fb1, fb2=fb2):
                    csl = slice(g0 * N, (g0 + 2) * N)
                    state = {}
                    steps = []

                    def mk_start():
                        state["pf"] = pfp.tile([128, 512], F32, tag="pf",
                                               name="pf")
                        state["Qv"] = Qbuf[:].rearrange("p (a b) -> p a b", b=50)
                        nc.tensor.matmul(state["pf"][:, 0:200], fn1avT,
                                         state["Qv"][:, :, 0:1],
                                         start=True, stop=False,
                                         skip_group_check=True)
                    steps.append(mk_start)
                    for c in range(1, 50):
                        def mk_mm(c=c):
                            nc.tensor.matmul(state["pf"][:, 0:200], fn1avT,
                                             state["Qv"][:, :, c:c + 1],
                                             start=False, stop=False,
                                             skip_group_check=True)
                        steps.append(mk_mm)

                    def mk_tail():
                        nc.tensor.matmul(state["pf"][:, 0:200], fn1xT,
                                         xt[0:d, csl],
                                         start=False, stop=True,
                                         skip_group_check=True)
                        y1 = y1pool.tile([128, 200], BF16, tag="y1")
                        nc.scalar.activation(y1[:], state["pf"][:, 0:200],
                                             AF.Tanh, bias=fb1)
                        pf2 = pfp.tile([od, 512], F32, tag="pf", name="pf2")
                        nc.tensor.matmul(pf2[:, 0:200], fn2T, y1[:],
                                         start=True, stop=True)
                        if blk < 2:
                            nc.scalar.activation(xnext[0:od, csl],
                                                 pf2[:, 0:200], AF.Tanh,
                                                 bias=fb2)
                        else:
                            nc.scalar.activation(ty[:, csl], pf2[:, 0:200],
                                                 AF.Tanh, bias=fb2)
                    steps.append(mk_tail)
                    return steps

                def build_m1(g_):
                    gsl_ = slice(g_ * N, (g_ + 1) * N)
                    # M1 = [x2-tiled; nrm; ones]  [d+2, PAIRS] bf16 -- all
                    # transfers have contiguous fastest-moving dims.
                    rep4 = rpool.tile([d, 400], BF16, tag="rep4")
                    nc.sync.dma_start(
                        rep4[:].rearrange("p (a b) -> p a b", a=4),
                        xt[0:d, gsl_].unsqueeze(1).broadcast_to([d, 4, N]),
                    )
                    M1 = mpool.tile([d + 2, PAIRS], BF16, tag="m1")
                    nc.sync.dma_start(
                        M1[0:d, :].rearrange("p (a b) -> p a b", a=25),
                        rep4[:].unsqueeze(1).broadcast_to([d, 25, 400]),
                    )
                    nc.sync.dma_start(
                        M1[d:d + 2, :],
                        din["nrmo"].ap()[2 * g_:2 * g_ + 2, :],
                    )
                    # A = W1a^T @ x  [128, 100] -- the x1 ("i") part of fe1,
                    # added per-node by the custom DVE op.
                    pa = ps2.tile([128, 512], F32, tag="p2", name="pa")
                    nc.tensor.matmul(pa[:, 0:100], s1a, xt[0:d, gsl_],
                                     start=True, stop=True)
                    asb = apool_a.tile([128, 100], F32, tag="asb")
                    nc.scalar.activation(asb[:], pa[:, 0:100], AF.Copy)
                    return M1, asb

                m1q = [build_m1(0), build_m1(1)]
                while deferred_loads:
                    tile_, dram_ = deferred_loads.pop(0)
                    nc.sync.dma_start(tile_[:], dram_.ap())

                for g in range(GPC):
                    M1, asb = m1q.pop(0)
                    if g + 2 < GPC:
                        m1q.append(build_m1(g + 2))

                    for u in range(UPG):
                        t1 = tpool.tile([128, 1000], BF16, tag="t1")
                        route_e = T1R[(g * UPG + u) % UPB] == "E"
                        for ci in range(2):
                            kc = (g * UPG + u) * 2 + ci
                            c0 = u * 1000 + ci * 500
                            i0 = u * UPG + ci * 5
                            p1 = ps1.tile([128, 512], F32, tag="p1")
                            nc.tensor.matmul(p1[:, 0:500], s1b,
                                             M1[:, c0:c0 + 500],
                                             start=True, stop=not route_e)
                            if route_e:
                                mv = xt[0:d, g * N + i0: g * N + i0 + 5] \
                                    .unsqueeze(2).broadcast_to([d, 5, N])
                                nc.tensor.matmul(p1[:, 0:500], s1a, mv,
                                                 start=False, stop=True)
                            pending_t1.append(
                                (p1, t1[:, ci * 500:(ci + 1) * 500], asb, kc))
                            if len(pending_t1) > T1_DELAY:
                                emit_t1(*pending_t1.pop(0))
                        pending_fe2.append((t1, g, u))
                        if len(pending_fe2) > FE2_DELAY:
                            emit_fe2(*pending_fe2.pop(0))
                        for _ in range(FN_SPREAD):
                            if pending_fn:
                                pending_fn.pop(0)()

                    if g % 2 == 1 and g >= 3:
                        pending_fn.extend(fn_pair_steps(
                            g - 3, qbufs.pop((g - 3) // 2), xnext=xnext))

                while pending_t1:
                    emit_t1(*pending_t1.pop(0))
                while pending_fe2:
                    emit_fe2(*pending_fe2.pop(0))
                while pending_trees:
                    emit_tree(*pending_trees.pop(0))
                while pending_fn:
                    pending_fn.pop(0)()
                for step in fn_pair_steps(GPC - 2, qbufs.pop((GPC - 2) // 2),
                                          xnext=xnext):
                    step()
                if blk < 2:
                    xt = xnext

            # ---------- final: sigmoid(mean over N) ----------
            red = xpool.tile([1, GPC], F32, tag="red")
            nc.vector.tensor_reduce(red[:], ty[:].rearrange("p (a b) -> p a b", a=GPC),
                                    axis=AX.X, op=ALU.add)
            osb = xpool.tile([1, GPC], F32, tag="osb")
            nc.scalar.activation(osb[:], red[:], AF.Sigmoid, scale=1.0 / N)
            nc.sync.dma_start(out_d.ap(), osb[:])

    nc.compile()
    return nc


def _host_prep(inputs):
    """Build per-core in_maps from full inputs."""
    x = np.asarray(inputs["x"], np.float32)          # [B, N, 2]

    bfl, nbf = _bf_pack_layout()
    f32l, nf32 = _f32_pack_layout()
    bfpack = np.zeros((128, nbf), np.float32)
    f32pack = np.zeros((128, nf32), np.float32)

    def put(name, arr):
        arr = np.asarray(arr, np.float32)
        for lay, pack in ((bfl, bfpack), (f32l, f32pack)):
            if name in lay:
                r, c, w = lay[name]
                assert arr.shape == (r, w), (name, arr.shape, (r, w))
                pack[0:r, c:c + w] = arr
                return
        raise KeyError(name)

    for i in range(3):
        d = NODE_SIZES[i]
        fe1w = np.asarray(inputs[f"fe1w{i}"], np.float32)   # [128, 2d+1]
        fe1b = np.asarray(inputs[f"fe1b{i}"], np.float32)
        fe2w = np.asarray(inputs[f"fe2w{i}"], np.float32)   # [128, 128]
        fe2b = np.asarray(inputs[f"fe2b{i}"], np.float32)
        fn1w = np.asarray(inputs[f"fn1w{i}"], np.float32)   # [128, 128+d]
        fn1b = np.asarray(inputs[f"fn1b{i}"], np.float32)
        fn2w = np.asarray(inputs[f"fn2w{i}"], np.float32)   # [od, 128]
        fn2b = np.asarray(inputs[f"fn2b{i}"], np.float32)
        if i == 0:
            perm = np.arange(d)
        else:
            # my x row order [y..., c0, c1] -> ref order [c0, c1, y...]
            perm = np.concatenate([np.arange(2, d), [0, 1]])
        W1a = fe1w[:, 0:d][:, perm].T                       # [d, 128]
        W1b = fe1w[:, d:2 * d][:, perm].T                   # [d, 128]
        s1b = np.concatenate(
            [W1b, fe1w[:, 2 * d].reshape(1, 128), fe1b.reshape(1, 128)], axis=0
        )
        put(f"s1b_{i}", s1b)
        put(f"s1a_{i}", W1a)
        put(f"fe2T{i}", fe2w.T)
        put(f"fn1xT{i}", fn1w[:, 128:][:, perm].T)
        put(f"fn1avT{i}", fn1w[:, :128].T)
        put(f"fn2T{i}", fn2w.T)
        put(f"b2_{i}", fe2b.reshape(128, 1))
        put(f"fb1_{i}", fn1b.reshape(128, 1))
        put(f"fb2_{i}", fn2b.reshape(FN2_OUT[i], 1))

    shared = {
        "bfpack": bfpack.astype(BF),
        "f32pack": f32pack,
    }

    in_maps = []
    for c in range(NCORES):
        xf = x[c * GPC:(c + 1) * GPC]                        # [16, 100, 2]
        xt0 = np.concatenate(
            [xf.transpose(2, 0, 1).reshape(2, COLS), np.ones((1, COLS), np.float32)],
            axis=0,
        )
        diff = xf[:, :, None, :] - xf[:, None, :, :]
        nrm = np.sqrt((diff * diff).sum(-1)).reshape(GPC, PAIRS)
        nrmo = np.empty((2 * GPC, PAIRS), np.float32)
        nrmo[0::2] = nrm
        nrmo[1::2] = 1.0
        m = dict(shared)
        m["xt0"] = xt0.astype(BF)
        m["nrmo"] = nrmo.astype(BF)
        in_maps.append(m)
    return in_maps


def kernel(**inputs):
    from concourse import bass_utils

    if "nc" not in _CACHE:
        _CACHE["nc"] = _build()
    nc = _CACHE["nc"]
    in_maps = _host_prep(inputs)
    res = bass_utils.run_bass_kernel_spmd(nc, in_maps, core_ids=list(range(NCORES)))
    out = np.concatenate(
        [np.asarray(res.results[c]["out"], np.float32).reshape(GPC, 1) for c in range(NCORES)],
        axis=0,
    )
    return out
